# revision 1
# baseline (speedup 1.0000x reference)
"""ChebNet (K=3, 3 layers) on 8 trn2 NeuronCores via Bass/Tile.

Strategy:
- Host: relabel nodes into 8 cores x 49 tiles x 128 slots, balancing per-tile
  in-edge counts; build per-core dst-sorted edge plans (gather indices packed
  for dma_gather, per-edge dst_local for on-device one-hot selectors).
- Device (SPMD, one program): each SpMM = dma_gather rows of the (replicated)
  feature table + one-hot selector matmuls accumulating per-tile segment sums
  in PSUM. Algebra folded so the 2nd propagation of each layer runs at the
  layer's output width: out = X(W0-W2) - A W1 + norm * segsum(G[src]), with
  A = prop(X), G = (norm^2 * A_agg) @ (2 W2). Halo exchange = AllGather of
  the G / H=(norm*out) tables between propagations.
"""
import sys
sys.path.insert(0, "/opt/trn_rl_repo")
import numpy as np

P = 128


class Cfg:
    def __init__(self, n_nodes, n_cores, groups, tpg, cl, ch, f_in, f_mid):
        self.n_nodes = n_nodes
        self.n_cores = n_cores
        self.groups = groups          # tile groups per core
        self.tpg = tpg                # tiles per group
        self.tpc = groups * tpg       # tiles per core
        self.cl = cl                  # low chunks per tile
        self.ch = ch                  # high chunks per tile
        self.nchk = cl + ch
        self.f_in = f_in              # layer-1 input width
        self.f_mid = f_mid            # exchanged-table width (64)
        self.nloc = self.tpc * P      # node slots per core
        self.nslot = n_cores * self.nloc
        self.lowb = (self.nslot // 2) // P * P  # low/high row split, tile-aligned
        assert self.lowb % P == 0 and self.lowb < 32768 + P
        # low/high boundary must be <= 32767+1 for int16 indexing
        assert self.lowb <= 32768 and self.nslot - self.lowb <= 32768


def full_cfg():
    return Cfg(n_nodes=50000, n_cores=8, groups=7, tpg=7, cl=9, ch=9,
               f_in=128, f_mid=64)


# ---------------------------------------------------------------- planner ---

def plan_graph(cfg, src, dst, rng_seed=0):
    """Assign nodes to (core, tile, slot); build per-core edge plans.

    Returns dict with:
      row_of[node] -> global table row; per-core idxL/idxH int16 arrays,
      dstsel fp32 arrays, per-core node lists.
    """
    N, NT = cfg.n_nodes, cfg.n_cores * cfg.tpc
    deg = np.bincount(dst, minlength=N).astype(np.int64)

    # --- greedy balance nodes into NT tiles (<=128 nodes each) by degree ---
    order = np.argsort(-deg, kind="stable")
    import heapq
    heap = [(0, t) for t in range(NT)]
    heapq.heapify(heap)
    tile_of = np.empty(N, np.int32)
    tile_cnt = np.zeros(NT, np.int32)
    tile_load = np.zeros(NT, np.int64)
    spill = []
    for v in order:
        while True:
            load, t = heapq.heappop(heap)
            if tile_cnt[t] < P:
                break
        tile_of[v] = t
        tile_cnt[t] += 1
        tile_load[t] += deg[v]
        if tile_cnt[t] < P:
            heapq.heappush(heap, (tile_load[t], t))

    # slot within tile
    slot_of = np.empty(N, np.int32)
    csl = np.zeros(NT, np.int32)
    for v in range(N):
        t = tile_of[v]
        slot_of[v] = csl[t]
        csl[t] += 1
    row_of = tile_of.astype(np.int64) * P + slot_of  # global table row

    # --- classify edges by src row region; per-(dsttile) low/high lists ---
    src_row = row_of[src]
    dst_row = row_of[dst]
    dst_tile = (dst_row // P).astype(np.int64)
    dst_loc = (dst_row % P).astype(np.int64)
    is_low = src_row < cfg.lowb

    # per-tile counts; repair pass: swap nodes between tiles in the same
    # region to fit (cl, ch) caps.
    capl, caph = cfg.cl * P, cfg.ch * P
    for _attempt in range(3):
        nl = np.bincount(dst_tile[is_low], minlength=NT)
        nh = np.bincount(dst_tile[~is_low], minlength=NT)
        over = (nl > capl) | (nh > caph)
        if not over.any():
            break
        # per-node low/high in-degree
        dl = np.bincount(dst[is_low], minlength=N)
        dh = np.bincount(dst[~is_low], minlength=N)
        lowtiles = cfg.lowb // P
        moved = 0
        for t in np.nonzero(over)[0]:
            region = range(0, lowtiles) if t < lowtiles else range(lowtiles, NT)
            nodes_t = np.nonzero(tile_of == t)[0]
            # try swapping heavy nodes out with light nodes elsewhere
            for v in nodes_t[np.argsort(-(dl[nodes_t] + dh[nodes_t]))]:
                if nl[t] <= capl and nh[t] <= caph:
                    break
                best = None
                for t2 in region:
                    if t2 == t:
                        continue
                    # find a light node in t2 to swap with v
                    nodes2 = np.nonzero(tile_of == t2)[0]
                    if len(nodes2) == 0:
                        continue
                    w = nodes2[np.argmin(dl[nodes2] + dh[nodes2])]
                    if (nl[t] - dl[v] + dl[w] <= capl and nh[t] - dh[v] + dh[w] <= caph
                            and nl[t2] - dl[w] + dl[v] <= capl
                            and nh[t2] - dh[w] + dh[v] <= caph):
                        best = (t2, w)
                        break
                if best is None:
                    continue
                t2, w = best
                tile_of[v], tile_of[w] = t2, t
                nl[t] += dl[w] - dl[v]; nh[t] += dh[w] - dh[v]
                nl[t2] += dl[v] - dl[w]; nh[t2] += dh[v] - dh[w]
                moved += 1
        # recompute rows after swaps
        slot_of = np.empty(N, np.int32)
        csl = np.zeros(NT, np.int32)
        for v in range(N):
            t = tile_of[v]
            slot_of[v] = csl[t]
            csl[t] += 1
        row_of = tile_of.astype(np.int64) * P + slot_of
        src_row = row_of[src]
        dst_row = row_of[dst]
        dst_tile = (dst_row // P).astype(np.int64)
        dst_loc = (dst_row % P).astype(np.int64)
        is_low = src_row < cfg.lowb
    nl = np.bincount(dst_tile[is_low], minlength=NT)
    nh = np.bincount(dst_tile[~is_low], minlength=NT)
    assert nl.max() <= capl and nh.max() <= caph, \
        f"tile overflow: nl.max={nl.max()}/{capl} nh.max={nh.max()}/{caph}"

    # --- per-core packed plans ---
    # order edges by (tile, region); within: sort by src row for DMA locality
    idxL = []
    idxH = []
    dsts = []
    ecl, ech = cfg.cl * P, cfg.ch * P
    # sort key: (tile, region [low first], src_row)
    key_region = (~is_low).astype(np.int64)
    order_all = np.lexsort((src_row, key_region, dst_tile))
    et = dst_tile[order_all]
    el = is_low[order_all]
    esrc = src_row[order_all]
    edst = dst_loc[order_all]
    sortkey = et * 2 + (1 - el.astype(np.int64))
    bounds = np.searchsorted(sortkey, np.arange(2 * NT + 1))
    for c in range(cfg.n_cores):
        t0 = c * cfg.tpc
        li = np.zeros((cfg.tpc, ecl), np.int64)          # gather idx (low)
        hi = np.zeros((cfg.tpc, ech), np.int64)
        dv = np.full((cfg.tpc, cfg.nchk * P), -1.0, np.float32)  # dst_local
        for ti in range(cfg.tpc):
            t = t0 + ti
            a, b = bounds[2 * t], bounds[2 * t + 1]
            sl, dl_ = esrc[a:b], edst[a:b]
            a, b = bounds[2 * t + 1], bounds[2 * t + 2]
            sh, dh_ = esrc[a:b] - cfg.lowb, edst[a:b]
            li[ti, :len(sl)] = sl
            hi[ti, :len(sh)] = sh
            # chunk layout: chunks 0..cl-1 low, cl..nchk-1 high; edge i ->
            # chunk i//P partition i%P, linear position == i
            dv[ti, :len(dl_)] = dl_
            dv[ti, cfg.cl * P:cfg.cl * P + len(dh_)] = dh_
        idxL.append(li)
        idxH.append(hi)
        dsts.append(dv)

    return dict(row_of=row_of, tile_of=tile_of, deg=deg,
                idxL=idxL, idxH=idxH, dstsel=dsts)


def pack_gather_idx(cfg, idx_core, chunks_per_tile):
    """[tpc, chunks*P] int64 -> dma_gather layout [128, total/16] int16,
    wrapped i -> [i%16, i//16], replicated across 8 gpsimd cores."""
    flat = idx_core.reshape(-1)  # tile-major edge stream
    n = len(flat)
    a = np.zeros((16, n // 16), np.int16)
    ii = np.arange(n)
    a[ii % 16, ii // 16] = flat.astype(np.int16)
    return np.tile(a, (8, 1))


def pack_dstsel(cfg, dv_core):
    """[tpc, nchk*P] -> [128, tpc*nchk] fp32: value for edge (chunk q, part p)
    at [p, q]."""
    tpc = dv_core.shape[0]
    dv = dv_core.reshape(tpc, cfg.nchk, P)      # [t, c, p]
    return dv.transpose(2, 0, 1).reshape(P, tpc * cfg.nchk).astype(np.float32)


# ---------------------------------------------------------------- builder ---

def build_kernel(cfg, use_coll=True, n_layers=3, n_phases=2):
    from concourse import bass, bacc, mybir, tile

    f32 = mybir.dt.float32
    NLOC, NSLOT = cfg.nloc, cfg.nslot
    TPC, GR, TPG, NCHK, CL, CH = cfg.tpc, cfg.groups, cfg.tpg, cfg.nchk, cfg.cl, cfg.ch
    FI, FM = cfg.f_in, cfg.f_mid
    LOWB = cfg.lowb
    nlow_cols = TPC * CL * P // 16
    nhigh_cols = TPC * CH * P // 16

    nc = bacc.Bacc("TRN2", target_bir_lowering=False, debug=False,
                   num_devices=cfg.n_cores)

    y1 = nc.declare_dram_parameter("y1", [NSLOT, FI], f32, isOutput=False)
    xloc = nc.declare_dram_parameter("xloc", [NLOC, FI], f32, isOutput=False)
    # plan: [iota(128) | ident(128) | dstsel(tpc*nchk)]
    plan = nc.declare_dram_parameter("plan", [P, 256 + TPC * NCHK], f32, isOutput=False)
    # nrm: [norm(tpc) | norm^2(tpc)]
    nrm_in = nc.declare_dram_parameter("nrm", [P, 2 * TPC], f32, isOutput=False)
    idxl_in = nc.declare_dram_parameter("idxl", [P, nlow_cols], mybir.dt.int16, isOutput=False)
    idxh_in = nc.declare_dram_parameter("idxh", [P, nhigh_cols], mybir.dt.int16, isOutput=False)
    # wts: 9 slots of [128, FM] (U1 V1 Z1 U2 V2 Z2 U3 V3 Z3) + 3 bias [128, FM]
    wts_in = nc.declare_dram_parameter("wts", [P, 12 * FM], f32, isOutput=False)
    outp = nc.declare_dram_parameter("out", [NLOC, FM], f32, isOutput=True)

    gloc = nc.dram_tensor("gloc", [NLOC, FM], f32)
    hloc = nc.dram_tensor("hloc", [NLOC, FM], f32)
    tabG = nc.dram_tensor("tabG", [NSLOT, FM], f32)
    tabH = nc.dram_tensor("tabH", [NSLOT, FM], f32)
    tabH2 = nc.dram_tensor("tabH2", [NSLOT, FM], f32)

    with tile.TileContext(nc) as tc:
        from contextlib import ExitStack
        ctx = ExitStack()
        res = ctx.enter_context(tc.tile_pool(name="resident", bufs=1))

        # ------- resident loads -------
        plan_t = res.tile([P, 256 + TPC * NCHK], f32)
        nrm_t = res.tile([P, 2 * TPC], f32)
        idxl_t = res.tile([P, nlow_cols], mybir.dt.int16)
        idxh_t = res.tile([P, nhigh_cols], mybir.dt.int16)
        wts_t = res.tile([P, 12 * FM], f32)
        nc.sync.dma_start(out=plan_t[:], in_=plan[:])
        nc.sync.dma_start(out=nrm_t[:], in_=nrm_in[:])
        nc.sync.dma_start(out=idxl_t[:], in_=idxl_in[:])
        nc.sync.dma_start(out=idxh_t[:], in_=idxh_in[:])
        nc.sync.dma_start(out=wts_t[:], in_=wts_in[:])
        iota_t = plan_t[:, 0:128]
        ident_t = plan_t[:, 128:256]
        dst_t = plan_t[:, 256:]
        norm_t = nrm_t[:, 0:TPC]
        norm2_t = nrm_t[:, TPC:]

        aloc = res.tile([P, TPC * FI], f32)     # A_local tiles (transposed-free layout)
        xcur = res.tile([P, TPC * FM], f32)     # next-layer local X (layers 2,3)

        def wslot(i, rows):
            return wts_t[0:rows, i * FM:(i + 1) * FM]

        # ------- spmm skeleton -------
        qctr = [0]

        def spmm(table, F, consumer, phase):
            """segment-sum over edges: consumer(g, t, psum_tile[P, F])."""
            subch = 8  # chunks per dma_gather call (SWDGE carveout: <=1024 idx)
            with tc.tile_pool(name=f"msg{phase}", bufs=2) as mp, \
                 tc.tile_pool(name=f"sel{phase}", bufs=2) as sp, \
                 tc.tile_pool(name=f"ps{phase}", bufs=2, space="PSUM") as pp:
                for g in range(GR):
                    # gather slabs for this group, lazily per sub-slab
                    slabs = {}

                    def ensure(stream, q):
                        # stream 0 = low, 1 = high
                        s = q // subch
                        key = (stream, s)
                        if key in slabs:
                            return slabs[key], q - s * subch
                        per_tile = CL if stream == 0 else CH
                        total = TPG * per_tile
                        lo = s * subch
                        hi = min(lo + subch, total)
                        cnt = hi - lo
                        buf = mp.tile([P, min(subch, total), F], f32, tag=f"m{stream}")
                        idx_res = idxl_t if stream == 0 else idxh_t
                        base_chunk = (g * TPG) * per_tile + lo
                        cols = slice(base_chunk * P // 16, (base_chunk + cnt) * P // 16)
                        src_ap = table[0:LOWB, :] if stream == 0 else table[LOWB:NSLOT, :]
                        nc.gpsimd.dma_gather(
                            out_ap=buf[:, 0:cnt, :], in_ap=src_ap,
                            idxs_ap=idx_res[:, cols],
                            num_idxs=cnt * P, num_idxs_reg=cnt * P, elem_size=F)
                        slabs[key] = buf
                        return buf, q - s * subch

                    for t in range(TPG):
                        gt = g * TPG + t
                        S = sp.tile([P, NCHK, P], f32, tag="S")
                        nc.vector.tensor_tensor(
                            out=S[:],
                            in0=dst_t[:, gt * NCHK:(gt + 1) * NCHK, None]
                                .broadcast_to([P, NCHK, P]),
                            in1=iota_t[:, None, :].broadcast_to([P, NCHK, P]),
                            op=mybir.AluOpType.is_equal)
                        ps = pp.tile([P, F], f32, space="PSUM", tag="acc")
                        for c in range(NCHK):
                            if c < CL:
                                buf, slot = ensure(0, t * CL + c)
                            else:
                                buf, slot = ensure(1, t * CH + (c - CL))
                            nc.tensor.matmul(out=ps[:], lhsT=S[:, c, :],
                                             rhs=buf[:, slot, :],
                                             start=(c == 0), stop=(c == NCHK - 1))
                        consumer(g, t, ps)

        # ------- layers -------
        for lay in range(n_layers):
            F_in = FI if lay == 0 else FM
            table = y1 if lay == 0 else (tabH if lay == 1 else tabH2)
            U = wslot(3 * lay + 0, F_in)
            V = wslot(3 * lay + 1, F_in)
            Z = wslot(3 * lay + 2, F_in)
            bias = wts_t[:, (9 + lay) * FM:(10 + lay) * FM]

            with tc.tile_pool(name=f"d1_{lay}", bufs=2) as dp, \
                 tc.tile_pool(name=f"dps1_{lay}", bufs=2, space="PSUM") as dpp:

                def consumer1(g, t, ps, lay=lay, F_in=F_in, Z=Z, dp=dp, dpp=dpp):
                    gt = g * TPG + t
                    # A_local = norm * agg (fp32, resident)
                    nc.scalar.activation(
                        out=aloc[:, gt * F_in:(gt + 1) * F_in], in_=ps[:],
                        func=mybir.ActivationFunctionType.Copy,
                        scale=norm_t[:, gt:gt + 1])
                    # gsrc = norm^2 * agg
                    gs = dp.tile([P, F_in], f32, tag="gs")
                    nc.scalar.activation(
                        out=gs[:], in_=ps[:],
                        func=mybir.ActivationFunctionType.Copy,
                        scale=norm2_t[:, gt:gt + 1])
                    # transpose
                    pt = dpp.tile([F_in, P], f32, space="PSUM", tag="pt")
                    nc.tensor.transpose(out=pt[:], in_=gs[:], identity=ident_t[:])
                    gst = dp.tile([F_in, P], f32, tag="gst")
                    nc.vector.tensor_copy(out=gst[:], in_=pt[:])
                    # G^T = Z^T @ gsrc^T
                    pg = dpp.tile([FM, P], f32, space="PSUM", tag="pg")
                    nc.tensor.matmul(out=pg[:], lhsT=Z, rhs=gst[:],
                                     start=True, stop=True)
                    gtt = dp.tile([FM, P], f32, tag="gtt")
                    nc.vector.tensor_copy(out=gtt[:], in_=pg[:])
                    # back to [P, FM]
                    pg2 = dpp.tile([P, FM], f32, space="PSUM", tag="pg2")
                    nc.tensor.transpose(out=pg2[:], in_=gtt[:],
                                        identity=ident_t[0:FM, 0:FM])
                    gsb = dp.tile([P, FM], f32, tag="gsb")
                    nc.scalar.copy(out=gsb[:], in_=pg2[:])
                    nc.sync.dma_start(out=gloc[gt * P:(gt + 1) * P, :], in_=gsb[:])

                spmm(table, F_in, consumer1, phase=f"a{lay}")

            if use_coll:
                nc.gpsimd.collective_compute(
                    "AllGather", bass.mybir.AluOpType.bypass,
                    replica_groups=[list(range(cfg.n_cores))],
                    ins=[gloc[:].opt()], outs=[tabG[:].opt()])
            else:
                nc.gpsimd.dma_start(out=tabG[0:NLOC, :], in_=gloc[:])

            if lay == n_layers - 1 and n_phases == 1:
                continue
            with tc.tile_pool(name=f"d2_{lay}", bufs=2) as dp2, \
                 tc.tile_pool(name=f"dps2_{lay}", bufs=2, space="PSUM") as dpp2:

                def consumer2(g, t, ps, lay=lay, F_in=F_in, U=U, V=V,
                              bias=bias, dp=dp2, dpp=dpp2):
                    gt = g * TPG + t
                    o1 = dp.tile([P, FM], f32, tag="o1")
                    nc.scalar.activation(
                        out=o1[:], in_=ps[:],
                        func=mybir.ActivationFunctionType.Copy,
                        scale=norm_t[:, gt:gt + 1])
                    # X U term
                    if lay == 0:
                        xv = dp.tile([P, FI], f32, tag="xv")
                        nc.sync.dma_start(out=xv[:],
                                          in_=xloc[gt * P:(gt + 1) * P, :])
                        xin = xv[:]
                    else:
                        xin = xcur[:, gt * FM:(gt + 1) * FM]
                    px = dpp.tile([F_in, P], f32, space="PSUM", tag="px")
                    nc.tensor.transpose(out=px[:], in_=xin, identity=ident_t[:])
                    xts = dp.tile([F_in, P], f32, tag="xts")
                    nc.vector.tensor_copy(out=xts[:], in_=px[:])
                    # A V term transpose
                    pa = dpp.tile([F_in, P], f32, space="PSUM", tag="px")
                    nc.tensor.transpose(out=pa[:],
                                        in_=aloc[:, gt * F_in:(gt + 1) * F_in],
                                        identity=ident_t[:])
                    ats = dp.tile([F_in, P], f32, tag="ats")
                    nc.vector.tensor_copy(out=ats[:], in_=pa[:])
                    po = dpp.tile([P, FM], f32, space="PSUM", tag="po")
                    nc.tensor.matmul(out=po[:], lhsT=xts[:], rhs=U,
                                     start=True, stop=False)
                    nc.tensor.matmul(out=po[:], lhsT=ats[:], rhs=V,
                                     start=False, stop=True)
                    # sum + bias
                    osum = dp.tile([P, FM], f32, tag="osum")
                    nc.vector.tensor_add(out=osum[:], in0=o1[:], in1=po[:])
                    if lay == 2:
                        ofin = dp.tile([P, FM], f32, tag="ofin")
                        nc.vector.tensor_add(out=ofin[:], in0=osum[:], in1=bias)
                        nc.sync.dma_start(out=outp[gt * P:(gt + 1) * P, :],
                                          in_=ofin[:])
                    else:
                        nc.vector.tensor_add(out=xcur[:, gt * FM:(gt + 1) * FM],
                                             in0=osum[:], in1=bias)
                        hv = dp.tile([P, FM], f32, tag="hv")
                        nc.scalar.activation(
                            out=hv[:], in_=xcur[:, gt * FM:(gt + 1) * FM],
                            func=mybir.ActivationFunctionType.Copy,
                            scale=norm_t[:, gt:gt + 1])
                        nc.sync.dma_start(out=hloc[gt * P:(gt + 1) * P, :],
                                          in_=hv[:])

                spmm(tabG, FM, consumer2, phase=f"b{lay}")

            if lay < 2:
                if use_coll:
                    nc.gpsimd.collective_compute(
                        "AllGather", bass.mybir.AluOpType.bypass,
                        replica_groups=[list(range(cfg.n_cores))],
                        ins=[hloc[:].opt()],
                        outs=[(tabH if lay == 0 else tabH2)[:].opt()])
                else:
                    nc.gpsimd.dma_start(
                        out=(tabH if lay == 0 else tabH2)[0:NLOC, :], in_=hloc[:])
        ctx.close()

    nc.compile()
    return nc


# ----------------------------------------------------------------- runner ---

def make_inputs(cfg, pl, features, src, dst, Ws, bs):
    """Build per-core in_maps. Ws = [W1, W2, W3] full blocks; bs = biases."""
    N = cfg.n_nodes
    deg = pl["deg"]
    norm = np.where(deg < 1, 1.0, deg).astype(np.float64) ** -0.5
    norm = norm.astype(np.float32)
    row_of = pl["row_of"]

    FI, FM = cfg.f_in, cfg.f_mid
    # y1 table (replicated): row_of[v] <- norm[v]*X[v]
    y1 = np.zeros((cfg.nslot, FI), np.float32)
    y1[row_of] = features * norm[:, None]

    # weights packed: per layer U=W0-W2, V=-W1, Z=2*W2 (padded to [128, FM])
    wts = np.zeros((P, 12 * FM), np.float32)
    for lay, W in enumerate(Ws):
        f_in = FI if lay == 0 else FM
        f_out = W.shape[1]
        W0, W1b, W2b = W[:f_in], W[f_in:2 * f_in], W[2 * f_in:]
        wts[0:f_in, (3 * lay) * FM:(3 * lay) * FM + f_out] = W0 - W2b
        wts[0:f_in, (3 * lay + 1) * FM:(3 * lay + 1) * FM + f_out] = -W1b
        wts[0:f_in, (3 * lay + 2) * FM:(3 * lay + 2) * FM + f_out] = 2.0 * W2b
        wts[:, (9 + lay) * FM:(9 + lay) * FM + f_out] = np.tile(bs[lay], (P, 1))

    iota = np.tile(np.arange(P, dtype=np.float32), (P, 1))
    ident = np.eye(P, dtype=np.float32)

    in_maps = []
    for c in range(cfg.n_cores):
        r0 = c * cfg.nloc
        # local node attrs in slot order
        xl = np.zeros((cfg.nloc, FI), np.float32)
        nl = np.ones(cfg.nloc, np.float32)
        mask = (row_of >= r0) & (row_of < r0 + cfg.nloc)
        vs = np.nonzero(mask)[0]
        xl[row_of[vs] - r0] = features[vs]
        nl[row_of[vs] - r0] = norm[vs]
        nrm = np.zeros((P, 2 * cfg.tpc), np.float32)
        nrm[:, :cfg.tpc] = nl.reshape(cfg.tpc, P).T
        nrm[:, cfg.tpc:] = (nl ** 2).reshape(cfg.tpc, P).T
        planv = np.concatenate([iota, ident, pack_dstsel(cfg, pl["dstsel"][c])], 1)
        in_maps.append({
            "y1": y1,
            "xloc": xl,
            "plan": planv.astype(np.float32),
            "nrm": nrm,
            "idxl": pack_gather_idx(cfg, pl["idxL"][c], cfg.cl),
            "idxh": pack_gather_idx(cfg, pl["idxH"][c], cfg.ch),
            "wts": wts,
        })
    return in_maps, norm


_CACHE = {}
TRACE = False
LAST_RESULT = None
LAST_INMAPS = None


def kernel(features, src, dst, W1, b1, W2, b2, W3, b3):
    from concourse.bass_utils import run_bass_kernel_spmd

    cfg = full_cfg()
    src = np.asarray(src).astype(np.int64)
    dst = np.asarray(dst).astype(np.int64)
    features = np.asarray(features, np.float32)

    pl = plan_graph(cfg, src, dst)
    in_maps, _ = make_inputs(cfg, pl, features, src, dst,
                             [np.asarray(W1, np.float32), np.asarray(W2, np.float32),
                              np.asarray(W3, np.float32)],
                             [np.asarray(b1, np.float32), np.asarray(b2, np.float32),
                              np.asarray(b3, np.float32)])

    if "nc" not in _CACHE:
        _CACHE["nc"] = build_kernel(cfg)
    nc = _CACHE["nc"]
    res = run_bass_kernel_spmd(nc, in_maps, core_ids=list(range(cfg.n_cores)),
                               trace=TRACE)
    global LAST_RESULT, LAST_INMAPS
    LAST_RESULT = res
    LAST_INMAPS = in_maps

    full = np.concatenate([res.results[c]["out"] for c in range(cfg.n_cores)], 0)
    out = full[pl["row_of"]][:, :W3.shape[1]]
    return out.astype(np.float32)



# revision 4
# speedup vs baseline: 23.3869x; 23.3869x over previous
"""ChebNet (K=3, 3 layers) on 8 trn2 NeuronCores via Bass/Tile.

Strategy:
- Host: relabel nodes into 8 cores x 49 tiles x 128 slots, balancing per-tile
  in-edge counts; build per-core dst-sorted edge plans (gather indices packed
  for dma_gather, per-edge dst_local for on-device one-hot selectors).
- Device (SPMD, one program): each SpMM = dma_gather rows of the (replicated)
  feature table + one-hot selector matmuls accumulating per-tile segment sums
  in PSUM. Algebra folded so the 2nd propagation of each layer runs at the
  layer's output width: out = X(W0-W2) - A W1 + norm * segsum(G[src]), with
  A = prop(X), G = (norm^2 * A_agg) @ (2 W2). Halo exchange = AllGather of
  the G / H=(norm*out) tables between propagations.
"""
import sys
sys.path.insert(0, "/opt/trn_rl_repo")
import numpy as np

P = 128


class Cfg:
    def __init__(self, n_nodes, n_cores, groups, tpg, cl, ch, f_in, f_mid):
        self.n_nodes = n_nodes
        self.n_cores = n_cores
        self.groups = groups          # tile groups per core
        self.tpg = tpg                # tiles per group
        self.tpc = groups * tpg       # tiles per core
        self.cl = cl                  # low chunks per tile
        self.ch = ch                  # high chunks per tile
        self.nchk = cl + ch
        self.f_in = f_in              # layer-1 input width
        self.f_mid = f_mid            # exchanged-table width (64)
        self.nloc = self.tpc * P      # node slots per core
        self.nslot = n_cores * self.nloc
        self.lowb = (self.nslot // 2) // P * P  # low/high row split, tile-aligned
        assert self.lowb % P == 0 and self.lowb < 32768 + P
        # low/high boundary must be <= 32767+1 for int16 indexing
        assert self.lowb <= 32768 and self.nslot - self.lowb <= 32768


def full_cfg():
    return Cfg(n_nodes=50000, n_cores=8, groups=7, tpg=7, cl=9, ch=9,
               f_in=128, f_mid=64)


# ---------------------------------------------------------------- planner ---

def plan_graph(cfg, src, dst, rng_seed=0):
    """Assign nodes to (core, tile, slot); build per-core edge plans.

    Returns dict with:
      row_of[node] -> global table row; per-core idxL/idxH int16 arrays,
      dstsel fp32 arrays, per-core node lists.
    """
    N, NT = cfg.n_nodes, cfg.n_cores * cfg.tpc
    deg = np.bincount(dst, minlength=N).astype(np.int64)

    # --- greedy balance nodes into NT tiles (<=128 nodes each) by degree ---
    order = np.argsort(-deg, kind="stable")
    import heapq
    heap = [(0, t) for t in range(NT)]
    heapq.heapify(heap)
    tile_of = np.empty(N, np.int32)
    tile_cnt = np.zeros(NT, np.int32)
    tile_load = np.zeros(NT, np.int64)
    spill = []
    for v in order:
        while True:
            load, t = heapq.heappop(heap)
            if tile_cnt[t] < P:
                break
        tile_of[v] = t
        tile_cnt[t] += 1
        tile_load[t] += deg[v]
        if tile_cnt[t] < P:
            heapq.heappush(heap, (tile_load[t], t))

    # slot within tile
    slot_of = np.empty(N, np.int32)
    csl = np.zeros(NT, np.int32)
    for v in range(N):
        t = tile_of[v]
        slot_of[v] = csl[t]
        csl[t] += 1
    row_of = tile_of.astype(np.int64) * P + slot_of  # global table row

    # --- classify edges by src row region; per-(dsttile) low/high lists ---
    src_row = row_of[src]
    dst_row = row_of[dst]
    dst_tile = (dst_row // P).astype(np.int64)
    dst_loc = (dst_row % P).astype(np.int64)
    is_low = src_row < cfg.lowb

    # per-tile counts; repair pass: swap nodes between tiles in the same
    # region to fit (cl, ch) caps.
    capl, caph = cfg.cl * P, cfg.ch * P
    for _attempt in range(3):
        nl = np.bincount(dst_tile[is_low], minlength=NT)
        nh = np.bincount(dst_tile[~is_low], minlength=NT)
        over = (nl > capl) | (nh > caph)
        if not over.any():
            break
        # per-node low/high in-degree
        dl = np.bincount(dst[is_low], minlength=N)
        dh = np.bincount(dst[~is_low], minlength=N)
        lowtiles = cfg.lowb // P
        moved = 0
        for t in np.nonzero(over)[0]:
            region = range(0, lowtiles) if t < lowtiles else range(lowtiles, NT)
            nodes_t = np.nonzero(tile_of == t)[0]
            # try swapping heavy nodes out with light nodes elsewhere
            for v in nodes_t[np.argsort(-(dl[nodes_t] + dh[nodes_t]))]:
                if nl[t] <= capl and nh[t] <= caph:
                    break
                best = None
                for t2 in region:
                    if t2 == t:
                        continue
                    # find a light node in t2 to swap with v
                    nodes2 = np.nonzero(tile_of == t2)[0]
                    if len(nodes2) == 0:
                        continue
                    w = nodes2[np.argmin(dl[nodes2] + dh[nodes2])]
                    if (nl[t] - dl[v] + dl[w] <= capl and nh[t] - dh[v] + dh[w] <= caph
                            and nl[t2] - dl[w] + dl[v] <= capl
                            and nh[t2] - dh[w] + dh[v] <= caph):
                        best = (t2, w)
                        break
                if best is None:
                    continue
                t2, w = best
                tile_of[v], tile_of[w] = t2, t
                nl[t] += dl[w] - dl[v]; nh[t] += dh[w] - dh[v]
                nl[t2] += dl[v] - dl[w]; nh[t2] += dh[v] - dh[w]
                moved += 1
        # recompute rows after swaps
        slot_of = np.empty(N, np.int32)
        csl = np.zeros(NT, np.int32)
        for v in range(N):
            t = tile_of[v]
            slot_of[v] = csl[t]
            csl[t] += 1
        row_of = tile_of.astype(np.int64) * P + slot_of
        src_row = row_of[src]
        dst_row = row_of[dst]
        dst_tile = (dst_row // P).astype(np.int64)
        dst_loc = (dst_row % P).astype(np.int64)
        is_low = src_row < cfg.lowb
    nl = np.bincount(dst_tile[is_low], minlength=NT)
    nh = np.bincount(dst_tile[~is_low], minlength=NT)
    assert nl.max() <= capl and nh.max() <= caph, \
        f"tile overflow: nl.max={nl.max()}/{capl} nh.max={nh.max()}/{caph}"

    # --- per-core packed plans ---
    # order edges by (tile, region); within: sort by src row for DMA locality
    idxL = []
    idxH = []
    dsts = []
    ecl, ech = cfg.cl * P, cfg.ch * P
    # sort key: (tile, region [low first], src_row)
    key_region = (~is_low).astype(np.int64)
    order_all = np.lexsort((src_row, key_region, dst_tile))
    et = dst_tile[order_all]
    el = is_low[order_all]
    esrc = src_row[order_all]
    edst = dst_loc[order_all]
    sortkey = et * 2 + (1 - el.astype(np.int64))
    bounds = np.searchsorted(sortkey, np.arange(2 * NT + 1))
    for c in range(cfg.n_cores):
        t0 = c * cfg.tpc
        li = np.zeros((cfg.tpc, ecl), np.int64)          # gather idx (low)
        hi = np.zeros((cfg.tpc, ech), np.int64)
        dv = np.full((cfg.tpc, cfg.nchk * P), -1.0, np.float32)  # dst_local
        for ti in range(cfg.tpc):
            t = t0 + ti
            a, b = bounds[2 * t], bounds[2 * t + 1]
            sl, dl_ = esrc[a:b], edst[a:b]
            a, b = bounds[2 * t + 1], bounds[2 * t + 2]
            sh, dh_ = esrc[a:b] - cfg.lowb, edst[a:b]
            li[ti, :len(sl)] = sl
            hi[ti, :len(sh)] = sh
            # chunk layout: chunks 0..cl-1 low, cl..nchk-1 high; edge i ->
            # chunk i//P partition i%P, linear position == i
            dv[ti, :len(dl_)] = dl_
            dv[ti, cfg.cl * P:cfg.cl * P + len(dh_)] = dh_
        idxL.append(li)
        idxH.append(hi)
        dsts.append(dv)

    return dict(row_of=row_of, tile_of=tile_of, deg=deg,
                idxL=idxL, idxH=idxH, dstsel=dsts)


def pack_gather_idx(cfg, idx_core, chunks_per_tile):
    """[tpc, chunks*P] int64 -> dma_gather layout [128, total/16] int16,
    wrapped i -> [i%16, i//16], replicated across 8 gpsimd cores."""
    flat = idx_core.reshape(-1)  # tile-major edge stream
    n = len(flat)
    a = np.zeros((16, n // 16), np.int16)
    ii = np.arange(n)
    a[ii % 16, ii // 16] = flat.astype(np.int16)
    return np.tile(a, (8, 1))


def pack_dstsel(cfg, dv_core):
    """[tpc, nchk*P] -> [128, tpc*nchk] fp32: value for edge (chunk q, part p)
    at [p, q]."""
    tpc = dv_core.shape[0]
    dv = dv_core.reshape(tpc, cfg.nchk, P)      # [t, c, p]
    return dv.transpose(2, 0, 1).reshape(P, tpc * cfg.nchk).astype(np.float32)


# ---------------------------------------------------------------- builder ---

def build_kernel(cfg, use_coll=True, n_layers=3, n_phases=2):
    from concourse import bass, bacc, mybir, tile

    f32 = mybir.dt.float32
    NLOC, NSLOT = cfg.nloc, cfg.nslot
    TPC, GR, TPG, NCHK, CL, CH = cfg.tpc, cfg.groups, cfg.tpg, cfg.nchk, cfg.cl, cfg.ch
    FI, FM = cfg.f_in, cfg.f_mid
    LOWB = cfg.lowb
    nlow_cols = TPC * CL * P // 16
    nhigh_cols = TPC * CH * P // 16

    nc = bacc.Bacc("TRN2", target_bir_lowering=False, debug=False,
                   num_devices=cfg.n_cores,
                   num_swdge_queues=4, dynamic_dma_scratch_size=32768)

    y1 = nc.declare_dram_parameter("y1", [NSLOT, FI], f32, isOutput=False)
    xloc = nc.declare_dram_parameter("xloc", [NLOC, FI], f32, isOutput=False)
    # plan: [iota(128) | ident(128) | dstsel(tpc*nchk)]
    plan = nc.declare_dram_parameter("plan", [P, 256 + TPC * NCHK], f32, isOutput=False)
    # nrm: [norm(tpc) | norm^2(tpc)]
    nrm_in = nc.declare_dram_parameter("nrm", [P, 2 * TPC], f32, isOutput=False)
    idxl_in = nc.declare_dram_parameter("idxl", [P, nlow_cols], mybir.dt.int16, isOutput=False)
    idxh_in = nc.declare_dram_parameter("idxh", [P, nhigh_cols], mybir.dt.int16, isOutput=False)
    # wts: 9 slots of [128, FM] (U1 V1 Z1 U2 V2 Z2 U3 V3 Z3) + 3 bias [128, FM]
    wts_in = nc.declare_dram_parameter("wts", [P, 12 * FM], f32, isOutput=False)
    outp = nc.declare_dram_parameter("out", [NLOC, FM], f32, isOutput=True)

    gloc = nc.dram_tensor("gloc", [NLOC, FM], f32)
    hloc = nc.dram_tensor("hloc", [NLOC, FM], f32)
    tabG = nc.dram_tensor("tabG", [NSLOT, FM], f32)
    tabH = nc.dram_tensor("tabH", [NSLOT, FM], f32)
    tabH2 = nc.dram_tensor("tabH2", [NSLOT, FM], f32)

    with tile.TileContext(nc) as tc:
        from contextlib import ExitStack
        ctx = ExitStack()
        res = ctx.enter_context(tc.tile_pool(name="resident", bufs=1))

        # ------- resident loads -------
        plan_t = res.tile([P, 256 + TPC * NCHK], f32)
        nrm_t = res.tile([P, 2 * TPC], f32)
        idxl_t = res.tile([P, nlow_cols], mybir.dt.int16)
        idxh_t = res.tile([P, nhigh_cols], mybir.dt.int16)
        wts_t = res.tile([P, 12 * FM], f32)
        nc.sync.dma_start(out=plan_t[:], in_=plan[:])
        nc.sync.dma_start(out=nrm_t[:], in_=nrm_in[:])
        nc.sync.dma_start(out=idxl_t[:], in_=idxl_in[:])
        nc.sync.dma_start(out=idxh_t[:], in_=idxh_in[:])
        nc.sync.dma_start(out=wts_t[:], in_=wts_in[:])
        iota_t = plan_t[:, 0:128]
        ident_t = plan_t[:, 128:256]
        dst_t = plan_t[:, 256:]
        norm_t = nrm_t[:, 0:TPC]
        norm2_t = nrm_t[:, TPC:]

        aloc = res.tile([P, TPC * FI], f32)     # A_local tiles (transposed-free layout)
        xcur = res.tile([P, TPC * FM], f32)     # next-layer local X (layers 2,3)

        def wslot(i, rows):
            return wts_t[0:rows, i * FM:(i + 1) * FM]

        # ------- spmm skeleton -------
        qctr = [0]

        def spmm(table, F, consumer, phase):
            """segment-sum over edges: consumer(g, t, psum_tile[P, F])."""
            subch = 8  # chunks per dma_gather call (1024 idx = half a ring)
            with tc.tile_pool(name=f"msg{phase}", bufs=4) as mp, \
                 tc.tile_pool(name=f"sel{phase}", bufs=2) as sp, \
                 tc.tile_pool(name=f"ps{phase}", bufs=2, space="PSUM") as pp:
                for g in range(GR):
                    # gather slabs for this group, lazily per sub-slab
                    slabs = {}

                    def ensure(stream, q):
                        # stream 0 = low, 1 = high
                        s = q // subch
                        key = (stream, s)
                        if key in slabs:
                            return slabs[key], q - s * subch
                        per_tile = CL if stream == 0 else CH
                        total = TPG * per_tile
                        lo = s * subch
                        hi = min(lo + subch, total)
                        cnt = hi - lo
                        buf = mp.tile([P, min(subch, total), F], f32, tag=f"m{stream}")
                        idx_res = idxl_t if stream == 0 else idxh_t
                        base_chunk = (g * TPG) * per_tile + lo
                        cols = slice(base_chunk * P // 16, (base_chunk + cnt) * P // 16)
                        src_ap = table[0:LOWB, :] if stream == 0 else table[LOWB:NSLOT, :]
                        nc.gpsimd.dma_gather(
                            out_ap=buf[:, 0:cnt, :], in_ap=src_ap,
                            idxs_ap=idx_res[:, cols],
                            num_idxs=cnt * P, num_idxs_reg=cnt * P, elem_size=F,
                            queue_num=qctr[0] % 4)
                        qctr[0] += 1
                        slabs[key] = buf
                        return buf, q - s * subch

                    for t in range(TPG):
                        gt = g * TPG + t
                        S = sp.tile([P, NCHK, P], f32, tag="S")
                        nc.vector.tensor_tensor(
                            out=S[:],
                            in0=dst_t[:, gt * NCHK:(gt + 1) * NCHK, None]
                                .broadcast_to([P, NCHK, P]),
                            in1=iota_t[:, None, :].broadcast_to([P, NCHK, P]),
                            op=mybir.AluOpType.is_equal)
                        ps = pp.tile([P, F], f32, space="PSUM", tag="acc")
                        for c in range(NCHK):
                            if c < CL:
                                buf, slot = ensure(0, t * CL + c)
                            else:
                                buf, slot = ensure(1, t * CH + (c - CL))
                            nc.tensor.matmul(out=ps[:], lhsT=S[:, c, :],
                                             rhs=buf[:, slot, :],
                                             start=(c == 0), stop=(c == NCHK - 1))
                        consumer(g, t, ps)

        # ------- layers -------
        for lay in range(n_layers):
            F_in = FI if lay == 0 else FM
            table = y1 if lay == 0 else (tabH if lay == 1 else tabH2)
            U = wslot(3 * lay + 0, F_in)
            V = wslot(3 * lay + 1, F_in)
            Z = wslot(3 * lay + 2, F_in)
            bias = wts_t[:, (9 + lay) * FM:(10 + lay) * FM]

            with tc.tile_pool(name=f"d1_{lay}", bufs=2) as dp, \
                 tc.tile_pool(name=f"dps1_{lay}", bufs=2, space="PSUM") as dpp:

                def consumer1(g, t, ps, lay=lay, F_in=F_in, Z=Z, dp=dp, dpp=dpp):
                    gt = g * TPG + t
                    # A_local = norm * agg (fp32, resident)
                    nc.scalar.activation(
                        out=aloc[:, gt * F_in:(gt + 1) * F_in], in_=ps[:],
                        func=mybir.ActivationFunctionType.Copy,
                        scale=norm_t[:, gt:gt + 1])
                    # gsrc = norm^2 * agg
                    gs = dp.tile([P, F_in], f32, tag="gs")
                    nc.scalar.activation(
                        out=gs[:], in_=ps[:],
                        func=mybir.ActivationFunctionType.Copy,
                        scale=norm2_t[:, gt:gt + 1])
                    # transpose
                    pt = dpp.tile([F_in, P], f32, space="PSUM", tag="pt")
                    nc.tensor.transpose(out=pt[:], in_=gs[:], identity=ident_t[:])
                    gst = dp.tile([F_in, P], f32, tag="gst")
                    nc.vector.tensor_copy(out=gst[:], in_=pt[:])
                    # G^T = Z^T @ gsrc^T
                    pg = dpp.tile([FM, P], f32, space="PSUM", tag="pg")
                    nc.tensor.matmul(out=pg[:], lhsT=Z, rhs=gst[:],
                                     start=True, stop=True)
                    gtt = dp.tile([FM, P], f32, tag="gtt")
                    nc.vector.tensor_copy(out=gtt[:], in_=pg[:])
                    # back to [P, FM]
                    pg2 = dpp.tile([P, FM], f32, space="PSUM", tag="pg2")
                    nc.tensor.transpose(out=pg2[:], in_=gtt[:],
                                        identity=ident_t[0:FM, 0:FM])
                    gsb = dp.tile([P, FM], f32, tag="gsb")
                    nc.scalar.copy(out=gsb[:], in_=pg2[:])
                    nc.sync.dma_start(out=gloc[gt * P:(gt + 1) * P, :], in_=gsb[:])

                spmm(table, F_in, consumer1, phase=f"a{lay}")

            if use_coll:
                nc.gpsimd.collective_compute(
                    "AllGather", bass.mybir.AluOpType.bypass,
                    replica_groups=[list(range(cfg.n_cores))],
                    ins=[gloc[:].opt()], outs=[tabG[:].opt()])
            else:
                nc.gpsimd.dma_start(out=tabG[0:NLOC, :], in_=gloc[:])

            if lay == n_layers - 1 and n_phases == 1:
                continue
            with tc.tile_pool(name=f"d2_{lay}", bufs=2) as dp2, \
                 tc.tile_pool(name=f"dps2_{lay}", bufs=2, space="PSUM") as dpp2:

                def consumer2(g, t, ps, lay=lay, F_in=F_in, U=U, V=V,
                              bias=bias, dp=dp2, dpp=dpp2):
                    gt = g * TPG + t
                    o1 = dp.tile([P, FM], f32, tag="o1")
                    nc.scalar.activation(
                        out=o1[:], in_=ps[:],
                        func=mybir.ActivationFunctionType.Copy,
                        scale=norm_t[:, gt:gt + 1])
                    # X U term
                    if lay == 0:
                        xv = dp.tile([P, FI], f32, tag="xv")
                        nc.sync.dma_start(out=xv[:],
                                          in_=xloc[gt * P:(gt + 1) * P, :])
                        xin = xv[:]
                    else:
                        xin = xcur[:, gt * FM:(gt + 1) * FM]
                    px = dpp.tile([F_in, P], f32, space="PSUM", tag="px")
                    nc.tensor.transpose(out=px[:], in_=xin, identity=ident_t[:])
                    xts = dp.tile([F_in, P], f32, tag="xts")
                    nc.vector.tensor_copy(out=xts[:], in_=px[:])
                    # A V term transpose
                    pa = dpp.tile([F_in, P], f32, space="PSUM", tag="px")
                    nc.tensor.transpose(out=pa[:],
                                        in_=aloc[:, gt * F_in:(gt + 1) * F_in],
                                        identity=ident_t[:])
                    ats = dp.tile([F_in, P], f32, tag="ats")
                    nc.vector.tensor_copy(out=ats[:], in_=pa[:])
                    po = dpp.tile([P, FM], f32, space="PSUM", tag="po")
                    nc.tensor.matmul(out=po[:], lhsT=xts[:], rhs=U,
                                     start=True, stop=False)
                    nc.tensor.matmul(out=po[:], lhsT=ats[:], rhs=V,
                                     start=False, stop=True)
                    # sum + bias
                    osum = dp.tile([P, FM], f32, tag="osum")
                    nc.vector.tensor_add(out=osum[:], in0=o1[:], in1=po[:])
                    if lay == 2:
                        ofin = dp.tile([P, FM], f32, tag="ofin")
                        nc.vector.tensor_add(out=ofin[:], in0=osum[:], in1=bias)
                        nc.sync.dma_start(out=outp[gt * P:(gt + 1) * P, :],
                                          in_=ofin[:])
                    else:
                        nc.vector.tensor_add(out=xcur[:, gt * FM:(gt + 1) * FM],
                                             in0=osum[:], in1=bias)
                        hv = dp.tile([P, FM], f32, tag="hv")
                        nc.scalar.activation(
                            out=hv[:], in_=xcur[:, gt * FM:(gt + 1) * FM],
                            func=mybir.ActivationFunctionType.Copy,
                            scale=norm_t[:, gt:gt + 1])
                        nc.sync.dma_start(out=hloc[gt * P:(gt + 1) * P, :],
                                          in_=hv[:])

                spmm(tabG, FM, consumer2, phase=f"b{lay}")

            if lay < 2:
                if use_coll:
                    nc.gpsimd.collective_compute(
                        "AllGather", bass.mybir.AluOpType.bypass,
                        replica_groups=[list(range(cfg.n_cores))],
                        ins=[hloc[:].opt()],
                        outs=[(tabH if lay == 0 else tabH2)[:].opt()])
                else:
                    nc.gpsimd.dma_start(
                        out=(tabH if lay == 0 else tabH2)[0:NLOC, :], in_=hloc[:])
        ctx.close()

    nc.compile()
    return nc


# ----------------------------------------------------------------- runner ---

def make_inputs(cfg, pl, features, src, dst, Ws, bs):
    """Build per-core in_maps. Ws = [W1, W2, W3] full blocks; bs = biases."""
    N = cfg.n_nodes
    deg = pl["deg"]
    norm = np.where(deg < 1, 1.0, deg).astype(np.float64) ** -0.5
    norm = norm.astype(np.float32)
    row_of = pl["row_of"]

    FI, FM = cfg.f_in, cfg.f_mid
    # y1 table (replicated): row_of[v] <- norm[v]*X[v]
    y1 = np.zeros((cfg.nslot, FI), np.float32)
    y1[row_of] = features * norm[:, None]

    # weights packed: per layer U=W0-W2, V=-W1, Z=2*W2 (padded to [128, FM])
    wts = np.zeros((P, 12 * FM), np.float32)
    for lay, W in enumerate(Ws):
        f_in = FI if lay == 0 else FM
        f_out = W.shape[1]
        W0, W1b, W2b = W[:f_in], W[f_in:2 * f_in], W[2 * f_in:]
        wts[0:f_in, (3 * lay) * FM:(3 * lay) * FM + f_out] = W0 - W2b
        wts[0:f_in, (3 * lay + 1) * FM:(3 * lay + 1) * FM + f_out] = -W1b
        wts[0:f_in, (3 * lay + 2) * FM:(3 * lay + 2) * FM + f_out] = 2.0 * W2b
        wts[:, (9 + lay) * FM:(9 + lay) * FM + f_out] = np.tile(bs[lay], (P, 1))

    iota = np.tile(np.arange(P, dtype=np.float32), (P, 1))
    ident = np.eye(P, dtype=np.float32)

    in_maps = []
    for c in range(cfg.n_cores):
        r0 = c * cfg.nloc
        # local node attrs in slot order
        xl = np.zeros((cfg.nloc, FI), np.float32)
        nl = np.ones(cfg.nloc, np.float32)
        mask = (row_of >= r0) & (row_of < r0 + cfg.nloc)
        vs = np.nonzero(mask)[0]
        xl[row_of[vs] - r0] = features[vs]
        nl[row_of[vs] - r0] = norm[vs]
        nrm = np.zeros((P, 2 * cfg.tpc), np.float32)
        nrm[:, :cfg.tpc] = nl.reshape(cfg.tpc, P).T
        nrm[:, cfg.tpc:] = (nl ** 2).reshape(cfg.tpc, P).T
        planv = np.concatenate([iota, ident, pack_dstsel(cfg, pl["dstsel"][c])], 1)
        in_maps.append({
            "y1": y1,
            "xloc": xl,
            "plan": planv.astype(np.float32),
            "nrm": nrm,
            "idxl": pack_gather_idx(cfg, pl["idxL"][c], cfg.cl),
            "idxh": pack_gather_idx(cfg, pl["idxH"][c], cfg.ch),
            "wts": wts,
        })
    return in_maps, norm


_CACHE = {}
TRACE = False
LAST_RESULT = None
LAST_INMAPS = None


def kernel(features, src, dst, W1, b1, W2, b2, W3, b3):
    from concourse.bass_utils import run_bass_kernel_spmd

    cfg = full_cfg()
    src = np.asarray(src).astype(np.int64)
    dst = np.asarray(dst).astype(np.int64)
    features = np.asarray(features, np.float32)

    pl = plan_graph(cfg, src, dst)
    in_maps, _ = make_inputs(cfg, pl, features, src, dst,
                             [np.asarray(W1, np.float32), np.asarray(W2, np.float32),
                              np.asarray(W3, np.float32)],
                             [np.asarray(b1, np.float32), np.asarray(b2, np.float32),
                              np.asarray(b3, np.float32)])

    if "nc" not in _CACHE:
        _CACHE["nc"] = build_kernel(cfg)
    nc = _CACHE["nc"]
    res = run_bass_kernel_spmd(nc, in_maps, core_ids=list(range(cfg.n_cores)),
                               trace=TRACE)
    global LAST_RESULT, LAST_INMAPS
    LAST_RESULT = res
    LAST_INMAPS = in_maps

    full = np.concatenate([res.results[c]["out"] for c in range(cfg.n_cores)], 0)
    out = full[pl["row_of"]][:, :W3.shape[1]]
    return out.astype(np.float32)



# revision 12
# speedup vs baseline: 35.7008x; 1.5265x over previous
"""ChebNet (K=3, 3 layers) on 8 trn2 NeuronCores via Bass/Tile.

Strategy:
- Host: relabel nodes into 8 cores x 49 tiles x 128 slots, balancing per-tile
  in-edge counts; build per-core dst-sorted edge plans (gather indices packed
  for dma_gather, per-edge dst_local for on-device one-hot selectors).
- Device (SPMD, one program): each SpMM = dma_gather rows of the (replicated)
  feature table + one-hot selector matmuls accumulating per-tile segment sums
  in PSUM. Algebra folded so the 2nd propagation of each layer runs at the
  layer's output width: out = X(W0-W2) - A W1 + norm * segsum(G[src]), with
  A = prop(X), G = (norm^2 * A_agg) @ (2 W2). Halo exchange = AllGather of
  the G / H=(norm*out) tables between propagations.
"""
import sys
sys.path.insert(0, "/opt/trn_rl_repo")
import numpy as np

P = 128


class Cfg:
    def __init__(self, n_nodes, n_cores, groups, tpg, cl, ch, f_in, f_mid):
        self.n_nodes = n_nodes
        self.n_cores = n_cores
        self.groups = groups          # tile groups per core
        self.tpg = tpg                # tiles per group
        self.tpc = groups * tpg       # tiles per core
        self.cl = cl                  # low chunks per tile
        self.ch = ch                  # high chunks per tile
        self.nchk = cl + ch
        self.f_in = f_in              # layer-1 input width
        self.f_mid = f_mid            # exchanged-table width (64)
        self.nloc = self.tpc * P      # node slots per core
        self.nslot = n_cores * self.nloc
        self.lowb = (self.nslot // 2) // P * P  # low/high row split, tile-aligned
        assert self.lowb % P == 0 and self.lowb < 32768 + P
        # low/high boundary must be <= 32767+1 for int16 indexing
        assert self.lowb <= 32768 and self.nslot - self.lowb <= 32768


def full_cfg():
    return Cfg(n_nodes=50000, n_cores=8, groups=7, tpg=7, cl=9, ch=9,
               f_in=128, f_mid=64)


# ---------------------------------------------------------------- planner ---

def plan_graph(cfg, src, dst, rng_seed=0):
    """Assign nodes to (core, tile, slot); build per-core edge plans.

    Returns dict with:
      row_of[node] -> global table row; per-core idxL/idxH int16 arrays,
      dstsel fp32 arrays, per-core node lists.
    """
    N, NT = cfg.n_nodes, cfg.n_cores * cfg.tpc
    deg = np.bincount(dst, minlength=N).astype(np.int64)

    # --- greedy balance nodes into NT tiles (<=128 nodes each) by degree ---
    order = np.argsort(-deg, kind="stable")
    import heapq
    heap = [(0, t) for t in range(NT)]
    heapq.heapify(heap)
    tile_of = np.empty(N, np.int32)
    tile_cnt = np.zeros(NT, np.int32)
    tile_load = np.zeros(NT, np.int64)
    spill = []
    for v in order:
        while True:
            load, t = heapq.heappop(heap)
            if tile_cnt[t] < P:
                break
        tile_of[v] = t
        tile_cnt[t] += 1
        tile_load[t] += deg[v]
        if tile_cnt[t] < P:
            heapq.heappush(heap, (tile_load[t], t))

    # slot within tile
    slot_of = np.empty(N, np.int32)
    csl = np.zeros(NT, np.int32)
    for v in range(N):
        t = tile_of[v]
        slot_of[v] = csl[t]
        csl[t] += 1
    row_of = tile_of.astype(np.int64) * P + slot_of  # global table row

    # --- classify edges by src row region; per-(dsttile) low/high lists ---
    src_row = row_of[src]
    dst_row = row_of[dst]
    dst_tile = (dst_row // P).astype(np.int64)
    dst_loc = (dst_row % P).astype(np.int64)
    is_low = src_row < cfg.lowb

    # per-tile counts; repair pass: swap nodes between tiles in the same
    # region to fit (cl, ch) caps.
    capl, caph = cfg.cl * P, cfg.ch * P
    for _attempt in range(3):
        nl = np.bincount(dst_tile[is_low], minlength=NT)
        nh = np.bincount(dst_tile[~is_low], minlength=NT)
        over = (nl > capl) | (nh > caph)
        if not over.any():
            break
        # per-node low/high in-degree
        dl = np.bincount(dst[is_low], minlength=N)
        dh = np.bincount(dst[~is_low], minlength=N)
        lowtiles = cfg.lowb // P
        moved = 0
        for t in np.nonzero(over)[0]:
            region = range(0, lowtiles) if t < lowtiles else range(lowtiles, NT)
            nodes_t = np.nonzero(tile_of == t)[0]
            # try swapping heavy nodes out with light nodes elsewhere
            for v in nodes_t[np.argsort(-(dl[nodes_t] + dh[nodes_t]))]:
                if nl[t] <= capl and nh[t] <= caph:
                    break
                best = None
                for t2 in region:
                    if t2 == t:
                        continue
                    # find a light node in t2 to swap with v
                    nodes2 = np.nonzero(tile_of == t2)[0]
                    if len(nodes2) == 0:
                        continue
                    w = nodes2[np.argmin(dl[nodes2] + dh[nodes2])]
                    if (nl[t] - dl[v] + dl[w] <= capl and nh[t] - dh[v] + dh[w] <= caph
                            and nl[t2] - dl[w] + dl[v] <= capl
                            and nh[t2] - dh[w] + dh[v] <= caph):
                        best = (t2, w)
                        break
                if best is None:
                    continue
                t2, w = best
                tile_of[v], tile_of[w] = t2, t
                nl[t] += dl[w] - dl[v]; nh[t] += dh[w] - dh[v]
                nl[t2] += dl[v] - dl[w]; nh[t2] += dh[v] - dh[w]
                moved += 1
        # recompute rows after swaps
        slot_of = np.empty(N, np.int32)
        csl = np.zeros(NT, np.int32)
        for v in range(N):
            t = tile_of[v]
            slot_of[v] = csl[t]
            csl[t] += 1
        row_of = tile_of.astype(np.int64) * P + slot_of
        src_row = row_of[src]
        dst_row = row_of[dst]
        dst_tile = (dst_row // P).astype(np.int64)
        dst_loc = (dst_row % P).astype(np.int64)
        is_low = src_row < cfg.lowb
    nl = np.bincount(dst_tile[is_low], minlength=NT)
    nh = np.bincount(dst_tile[~is_low], minlength=NT)
    assert nl.max() <= capl and nh.max() <= caph, \
        f"tile overflow: nl.max={nl.max()}/{capl} nh.max={nh.max()}/{caph}"

    # --- per-core packed plans ---
    # order edges by (tile, region); within: sort by src row for DMA locality
    idxL = []
    idxH = []
    dsts = []
    ecl, ech = cfg.cl * P, cfg.ch * P
    # sort key: (tile, region [low first], src_row)
    key_region = (~is_low).astype(np.int64)
    order_all = np.lexsort((src_row, key_region, dst_tile))
    et = dst_tile[order_all]
    el = is_low[order_all]
    esrc = src_row[order_all]
    edst = dst_loc[order_all]
    sortkey = et * 2 + (1 - el.astype(np.int64))
    bounds = np.searchsorted(sortkey, np.arange(2 * NT + 1))
    for c in range(cfg.n_cores):
        t0 = c * cfg.tpc
        li = np.zeros((cfg.tpc, ecl), np.int64)          # gather idx (low)
        hi = np.zeros((cfg.tpc, ech), np.int64)
        dv = np.full((cfg.tpc, cfg.nchk * P), -1.0, np.float32)  # dst_local
        for ti in range(cfg.tpc):
            t = t0 + ti
            a, b = bounds[2 * t], bounds[2 * t + 1]
            sl, dl_ = esrc[a:b], edst[a:b]
            a, b = bounds[2 * t + 1], bounds[2 * t + 2]
            sh, dh_ = esrc[a:b] - cfg.lowb, edst[a:b]
            li[ti, :len(sl)] = sl
            hi[ti, :len(sh)] = sh
            # chunk layout: chunks 0..cl-1 low, cl..nchk-1 high; edge i ->
            # chunk i//P partition i%P, linear position == i
            dv[ti, :len(dl_)] = dl_
            dv[ti, cfg.cl * P:cfg.cl * P + len(dh_)] = dh_
        idxL.append(li)
        idxH.append(hi)
        dsts.append(dv)

    return dict(row_of=row_of, tile_of=tile_of, deg=deg,
                idxL=idxL, idxH=idxH, dstsel=dsts)


def pack_gather_idx(cfg, idx_core, chunks_per_tile):
    """[tpc, chunks*P] int64 -> dma_gather layout [128, total/16] int16,
    wrapped i -> [i%16, i//16], replicated across 8 gpsimd cores."""
    flat = idx_core.reshape(-1)  # tile-major edge stream
    n = len(flat)
    a = np.zeros((16, n // 16), np.int16)
    ii = np.arange(n)
    a[ii % 16, ii // 16] = flat.astype(np.int16)
    return np.tile(a, (8, 1))


def pack_dstsel(cfg, dv_core):
    """[tpc, nchk*P] -> [128, tpc*nchk] fp32: value for edge (chunk q, part p)
    at [p, q]."""
    tpc = dv_core.shape[0]
    dv = dv_core.reshape(tpc, cfg.nchk, P)      # [t, c, p]
    return dv.transpose(2, 0, 1).reshape(P, tpc * cfg.nchk).astype(np.float32)


# ---------------------------------------------------------------- builder ---

def build_kernel(cfg, use_coll=True, n_layers=3, n_phases=2):
    from concourse import bass, bacc, mybir, tile

    f32 = mybir.dt.float32
    bf16 = mybir.dt.bfloat16
    NLOC, NSLOT = cfg.nloc, cfg.nslot
    TPC, GR, TPG, NCHK, CL, CH = cfg.tpc, cfg.groups, cfg.tpg, cfg.nchk, cfg.cl, cfg.ch
    FI, FM = cfg.f_in, cfg.f_mid
    LOWB = cfg.lowb
    nlow_cols = TPC * CL * P // 16
    nhigh_cols = TPC * CH * P // 16

    nc = bacc.Bacc("TRN2", target_bir_lowering=False, debug=False,
                   num_devices=cfg.n_cores,
                   num_swdge_queues=4, dynamic_dma_scratch_size=32768)

    y1 = nc.declare_dram_parameter("y1", [NSLOT, FI], bf16, isOutput=False)
    xloc = nc.declare_dram_parameter("xloc", [NLOC, FI], f32, isOutput=False)
    # plan: [iota(128) | ident(128) | dstsel(tpc*nchk)]
    plan = nc.declare_dram_parameter("plan", [P, 256 + TPC * NCHK], f32, isOutput=False)
    # nrm: [norm(tpc) | norm^2(tpc)]
    nrm_in = nc.declare_dram_parameter("nrm", [P, 2 * TPC], f32, isOutput=False)
    idxl_in = nc.declare_dram_parameter("idxl", [P, nlow_cols], mybir.dt.int16, isOutput=False)
    idxh_in = nc.declare_dram_parameter("idxh", [P, nhigh_cols], mybir.dt.int16, isOutput=False)
    # wts: 9 slots of [128, FM] (U1 V1 Z1 U2 V2 Z2 U3 V3 Z3) + 3 bias [128, FM]
    wts_in = nc.declare_dram_parameter("wts", [P, 12 * FM], f32, isOutput=False)
    outp = nc.declare_dram_parameter("out", [NLOC, FM], f32, isOutput=True)

    # bf16 tables padded to 128 cols so dma_gather's 256B-min elem works;
    # data lives in cols 0:FM, upper cols are never read.
    gloc = nc.dram_tensor("gloc", [NLOC, P], bf16)
    hloc = nc.dram_tensor("hloc", [NLOC, P], bf16)
    tabG = nc.dram_tensor("tabG", [NSLOT, P], bf16)
    tabH = nc.dram_tensor("tabH", [NSLOT, P], bf16)
    tabH2 = nc.dram_tensor("tabH2", [NSLOT, P], bf16)

    with tile.TileContext(nc) as tc:
        from contextlib import ExitStack
        ctx = ExitStack()
        res = ctx.enter_context(tc.tile_pool(name="resident", bufs=1))

        # ------- resident loads -------
        plan_t = res.tile([P, 256 + TPC * NCHK], f32)
        nrm_t = res.tile([P, 2 * TPC], f32)
        idxl_t = res.tile([P, nlow_cols], mybir.dt.int16)
        idxh_t = res.tile([P, nhigh_cols], mybir.dt.int16)
        wts_t = res.tile([P, 12 * FM], f32)
        nc.sync.dma_start(out=plan_t[:], in_=plan[:])
        nc.sync.dma_start(out=nrm_t[:], in_=nrm_in[:])
        nc.sync.dma_start(out=idxl_t[:], in_=idxl_in[:])
        nc.sync.dma_start(out=idxh_t[:], in_=idxh_in[:])
        nc.sync.dma_start(out=wts_t[:], in_=wts_in[:])
        iota_t = plan_t[:, 0:128]
        ident_t = plan_t[:, 128:256]
        dst_t = plan_t[:, 256:]
        norm_t = nrm_t[:, 0:TPC]
        norm2_t = nrm_t[:, TPC:]

        aloc = res.tile([P, TPC * FI], f32)     # A_local tiles (transposed-free layout)
        xcur = res.tile([P, TPC * FM], f32)     # next-layer local X (layers 2,3)

        def wslot(i, rows):
            return wts_t[0:rows, i * FM:(i + 1) * FM]

        # ------- spmm skeleton -------
        qctr = [0]

        def spmm(table, F, consumer, phase):
            """segment-sum over edges: consumer(g, t, psum_tile[P, F]).

            Tables are bf16 padded to 128 cols (256B rows — the dma_gather
            minimum elem); only cols 0:F are consumed by the matmuls."""
            subch = 8  # chunks per dma_gather call (1024 idx = half a ring)
            with tc.tile_pool(name=f"msg{phase}", bufs=4) as mp, \
                 tc.tile_pool(name=f"sel{phase}", bufs=2) as sp, \
                 tc.tile_pool(name=f"ps{phase}", bufs=2, space="PSUM") as pp:
                for g in range(GR):
                    # gather slabs for this group, lazily per sub-slab
                    slabs = {}

                    def ensure(stream, q):
                        # stream 0 = low, 1 = high
                        s = q // subch
                        key = (stream, s)
                        if key in slabs:
                            return slabs[key], q - s * subch
                        per_tile = CL if stream == 0 else CH
                        total = TPG * per_tile
                        lo = s * subch
                        hi = min(lo + subch, total)
                        cnt = hi - lo
                        buf = mp.tile([P, min(subch, total), P], bf16,
                                      tag=f"m{stream}")
                        idx_res = idxl_t if stream == 0 else idxh_t
                        base_chunk = (g * TPG) * per_tile + lo
                        cols = slice(base_chunk * P // 16, (base_chunk + cnt) * P // 16)
                        src_ap = table[0:LOWB, :] if stream == 0 else table[LOWB:NSLOT, :]
                        nc.gpsimd.dma_gather(
                            out_ap=buf[:, 0:cnt, :], in_ap=src_ap,
                            idxs_ap=idx_res[:, cols],
                            num_idxs=cnt * P, num_idxs_reg=cnt * P, elem_size=P,
                            queue_num=qctr[0] % 4)
                        qctr[0] += 1
                        slabs[key] = buf
                        return buf, q - s * subch

                    for t in range(TPG):
                        gt = g * TPG + t
                        S = sp.tile([P, NCHK, P], bf16, tag="S")
                        nc.vector.tensor_tensor(
                            out=S[:],
                            in0=dst_t[:, gt * NCHK:(gt + 1) * NCHK, None]
                                .broadcast_to([P, NCHK, P]),
                            in1=iota_t[:, None, :].broadcast_to([P, NCHK, P]),
                            op=mybir.AluOpType.is_equal)
                        ps = pp.tile([P, F], f32, space="PSUM", tag="acc")
                        for c in range(NCHK):
                            if c < CL:
                                buf, slot = ensure(0, t * CL + c)
                            else:
                                buf, slot = ensure(1, t * CH + (c - CL))
                            nc.tensor.matmul(out=ps[:], lhsT=S[:, c, :],
                                             rhs=buf[:, slot, 0:F],
                                             start=(c == 0), stop=(c == NCHK - 1))
                        consumer(g, t, ps)

        # ------- layers -------
        for lay in range(n_layers):
            F_in = FI if lay == 0 else FM
            table = y1 if lay == 0 else (tabH if lay == 1 else tabH2)
            U = wslot(3 * lay + 0, F_in)
            V = wslot(3 * lay + 1, F_in)
            Z = wslot(3 * lay + 2, F_in)
            bias = wts_t[:, (9 + lay) * FM:(10 + lay) * FM]

            with tc.tile_pool(name=f"d1_{lay}", bufs=2) as dp, \
                 tc.tile_pool(name=f"dps1_{lay}", bufs=2, space="PSUM") as dpp:

                def consumer1(g, t, ps, lay=lay, F_in=F_in, Z=Z, dp=dp, dpp=dpp):
                    gt = g * TPG + t
                    # A_local = norm * agg (fp32, resident)
                    nc.scalar.activation(
                        out=aloc[:, gt * F_in:(gt + 1) * F_in], in_=ps[:],
                        func=mybir.ActivationFunctionType.Copy,
                        scale=norm_t[:, gt:gt + 1])
                    # gsrc = norm^2 * agg
                    gs = dp.tile([P, F_in], f32, tag="gs")
                    nc.scalar.activation(
                        out=gs[:], in_=ps[:],
                        func=mybir.ActivationFunctionType.Copy,
                        scale=norm2_t[:, gt:gt + 1])
                    # transpose
                    pt = dpp.tile([F_in, P], f32, space="PSUM", tag="pt")
                    nc.tensor.transpose(out=pt[:], in_=gs[:], identity=ident_t[:])
                    gst = dp.tile([F_in, P], f32, tag="gst")
                    nc.vector.tensor_copy(out=gst[:], in_=pt[:])
                    # G^T = Z^T @ gsrc^T
                    pg = dpp.tile([FM, P], f32, space="PSUM", tag="pg")
                    nc.tensor.matmul(out=pg[:], lhsT=Z, rhs=gst[:],
                                     start=True, stop=True)
                    gtt = dp.tile([FM, P], f32, tag="gtt")
                    nc.vector.tensor_copy(out=gtt[:], in_=pg[:])
                    # back to [P, FM]
                    pg2 = dpp.tile([P, FM], f32, space="PSUM", tag="pg2")
                    nc.tensor.transpose(out=pg2[:], in_=gtt[:],
                                        identity=ident_t[0:FM, 0:FM])
                    gsb = dp.tile([P, FM], bf16, tag="gsb")
                    nc.scalar.copy(out=gsb[:], in_=pg2[:])
                    nc.sync.dma_start(out=gloc[gt * P:(gt + 1) * P, 0:FM],
                                      in_=gsb[:])

                spmm(table, F_in, consumer1, phase=f"a{lay}")

            if use_coll:
                nc.gpsimd.collective_compute(
                    "AllGather", bass.mybir.AluOpType.bypass,
                    replica_groups=[list(range(cfg.n_cores))],
                    ins=[gloc[:].opt()], outs=[tabG[:].opt()])
            else:
                nc.gpsimd.dma_start(out=tabG[0:NLOC, :], in_=gloc[:])

            if lay == n_layers - 1 and n_phases == 1:
                continue
            with tc.tile_pool(name=f"d2_{lay}", bufs=2) as dp2, \
                 tc.tile_pool(name=f"dps2_{lay}", bufs=2, space="PSUM") as dpp2:

                def consumer2(g, t, ps, lay=lay, F_in=F_in, U=U, V=V,
                              bias=bias, dp=dp2, dpp=dpp2):
                    gt = g * TPG + t
                    o1 = dp.tile([P, FM], f32, tag="o1")
                    nc.scalar.activation(
                        out=o1[:], in_=ps[:],
                        func=mybir.ActivationFunctionType.Copy,
                        scale=norm_t[:, gt:gt + 1])
                    # X U term
                    if lay == 0:
                        xv = dp.tile([P, FI], f32, tag="xv")
                        nc.sync.dma_start(out=xv[:],
                                          in_=xloc[gt * P:(gt + 1) * P, :])
                        xin = xv[:]
                    else:
                        xin = xcur[:, gt * FM:(gt + 1) * FM]
                    px = dpp.tile([F_in, P], f32, space="PSUM", tag="px")
                    nc.tensor.transpose(out=px[:], in_=xin, identity=ident_t[:])
                    xts = dp.tile([F_in, P], f32, tag="xts")
                    nc.vector.tensor_copy(out=xts[:], in_=px[:])
                    # A V term transpose
                    pa = dpp.tile([F_in, P], f32, space="PSUM", tag="px")
                    nc.tensor.transpose(out=pa[:],
                                        in_=aloc[:, gt * F_in:(gt + 1) * F_in],
                                        identity=ident_t[:])
                    ats = dp.tile([F_in, P], f32, tag="ats")
                    nc.vector.tensor_copy(out=ats[:], in_=pa[:])
                    po = dpp.tile([P, FM], f32, space="PSUM", tag="po")
                    nc.tensor.matmul(out=po[:], lhsT=xts[:], rhs=U,
                                     start=True, stop=False)
                    nc.tensor.matmul(out=po[:], lhsT=ats[:], rhs=V,
                                     start=False, stop=True)
                    # sum + bias
                    osum = dp.tile([P, FM], f32, tag="osum")
                    nc.vector.tensor_add(out=osum[:], in0=o1[:], in1=po[:])
                    if lay == 2:
                        ofin = dp.tile([P, FM], f32, tag="ofin")
                        nc.vector.tensor_add(out=ofin[:], in0=osum[:], in1=bias)
                        nc.sync.dma_start(out=outp[gt * P:(gt + 1) * P, :],
                                          in_=ofin[:])
                    else:
                        nc.vector.tensor_add(out=xcur[:, gt * FM:(gt + 1) * FM],
                                             in0=osum[:], in1=bias)
                        hv = dp.tile([P, FM], bf16, tag="hv")
                        nc.scalar.activation(
                            out=hv[:], in_=xcur[:, gt * FM:(gt + 1) * FM],
                            func=mybir.ActivationFunctionType.Copy,
                            scale=norm_t[:, gt:gt + 1])
                        nc.sync.dma_start(out=hloc[gt * P:(gt + 1) * P, 0:FM],
                                          in_=hv[:])

                spmm(tabG, FM, consumer2, phase=f"b{lay}")

            if lay < 2:
                if use_coll:
                    nc.gpsimd.collective_compute(
                        "AllGather", bass.mybir.AluOpType.bypass,
                        replica_groups=[list(range(cfg.n_cores))],
                        ins=[hloc[:].opt()],
                        outs=[(tabH if lay == 0 else tabH2)[:].opt()])
                else:
                    nc.gpsimd.dma_start(
                        out=(tabH if lay == 0 else tabH2)[0:NLOC, :], in_=hloc[:])
        ctx.close()

    nc.compile()
    return nc


# ----------------------------------------------------------------- runner ---

def make_inputs(cfg, pl, features, src, dst, Ws, bs):
    """Build per-core in_maps. Ws = [W1, W2, W3] full blocks; bs = biases."""
    N = cfg.n_nodes
    deg = pl["deg"]
    norm = np.where(deg < 1, 1.0, deg).astype(np.float64) ** -0.5
    norm = norm.astype(np.float32)
    row_of = pl["row_of"]

    FI, FM = cfg.f_in, cfg.f_mid
    # y1 table (replicated, bf16): row_of[v] <- norm[v]*X[v]
    import ml_dtypes
    y1 = np.zeros((cfg.nslot, FI), np.float32)
    y1[row_of] = features * norm[:, None]
    y1 = y1.astype(ml_dtypes.bfloat16)

    # weights packed: per layer U=W0-W2, V=-W1, Z=2*W2 (padded to [128, FM])
    wts = np.zeros((P, 12 * FM), np.float32)
    for lay, W in enumerate(Ws):
        f_in = FI if lay == 0 else FM
        f_out = W.shape[1]
        W0, W1b, W2b = W[:f_in], W[f_in:2 * f_in], W[2 * f_in:]
        wts[0:f_in, (3 * lay) * FM:(3 * lay) * FM + f_out] = W0 - W2b
        wts[0:f_in, (3 * lay + 1) * FM:(3 * lay + 1) * FM + f_out] = -W1b
        wts[0:f_in, (3 * lay + 2) * FM:(3 * lay + 2) * FM + f_out] = 2.0 * W2b
        wts[:, (9 + lay) * FM:(9 + lay) * FM + f_out] = np.tile(bs[lay], (P, 1))

    iota = np.tile(np.arange(P, dtype=np.float32), (P, 1))
    ident = np.eye(P, dtype=np.float32)

    in_maps = []
    for c in range(cfg.n_cores):
        r0 = c * cfg.nloc
        # local node attrs in slot order
        xl = np.zeros((cfg.nloc, FI), np.float32)
        nl = np.ones(cfg.nloc, np.float32)
        mask = (row_of >= r0) & (row_of < r0 + cfg.nloc)
        vs = np.nonzero(mask)[0]
        xl[row_of[vs] - r0] = features[vs]
        nl[row_of[vs] - r0] = norm[vs]
        nrm = np.zeros((P, 2 * cfg.tpc), np.float32)
        nrm[:, :cfg.tpc] = nl.reshape(cfg.tpc, P).T
        nrm[:, cfg.tpc:] = (nl ** 2).reshape(cfg.tpc, P).T
        planv = np.concatenate([iota, ident, pack_dstsel(cfg, pl["dstsel"][c])], 1)
        in_maps.append({
            "y1": y1,
            "xloc": xl,
            "plan": planv.astype(np.float32),
            "nrm": nrm,
            "idxl": pack_gather_idx(cfg, pl["idxL"][c], cfg.cl),
            "idxh": pack_gather_idx(cfg, pl["idxH"][c], cfg.ch),
            "wts": wts,
        })
    return in_maps, norm


_CACHE = {}
TRACE = False
LAST_RESULT = None
LAST_INMAPS = None


def kernel(features, src, dst, W1, b1, W2, b2, W3, b3):
    from concourse.bass_utils import run_bass_kernel_spmd

    cfg = full_cfg()
    src = np.asarray(src).astype(np.int64)
    dst = np.asarray(dst).astype(np.int64)
    features = np.asarray(features, np.float32)

    pl = plan_graph(cfg, src, dst)
    in_maps, _ = make_inputs(cfg, pl, features, src, dst,
                             [np.asarray(W1, np.float32), np.asarray(W2, np.float32),
                              np.asarray(W3, np.float32)],
                             [np.asarray(b1, np.float32), np.asarray(b2, np.float32),
                              np.asarray(b3, np.float32)])

    if "nc" not in _CACHE:
        _CACHE["nc"] = build_kernel(cfg)
    nc = _CACHE["nc"]
    res = run_bass_kernel_spmd(nc, in_maps, core_ids=list(range(cfg.n_cores)),
                               trace=TRACE)
    global LAST_RESULT, LAST_INMAPS
    LAST_RESULT = res
    LAST_INMAPS = in_maps

    full = np.concatenate([res.results[c]["out"] for c in range(cfg.n_cores)], 0)
    out = full[pl["row_of"]][:, :W3.shape[1]]
    return out.astype(np.float32)

